# revision 2
# baseline (speedup 1.0000x reference)
"""3-layer GAT + mean-pool + linear head on 8 Trainium2 NeuronCores. v2.

Changes vs baseline:
  - dst-row gather eliminated: per-edge a_dst is expanded on-chip with a
    host-shipped fp8 transposed indicator indT[n,e] per 128-edge tile and a
    1-column PE matmul against the slot's a_dst column (stashed at table
    build time).
  - src gathers merged into <=1024-descriptor calls (the deployed ucode
    SWDGE ring caps at 1024 descriptors), GB=5 slots per gather batch.
  - ones-column memset removed; the softmax denominator comes from a second
    PE matmul with a constant ones vector as rhs.
  - xT loaded in per-batch chunks instead of one persistent 50KB tile.
"""

import sys

for _p in ("/opt/trn_rl_repo",):
    if _p not in sys.path:
        sys.path.insert(0, _p)

import numpy as np
import ml_dtypes

import concourse.bacc as bacc
import concourse.bass as bass
import concourse.tile as tile
from concourse import bass_utils, mybir

# Problem constants (hardcoded per spec)
N = 100_000
E = 1_600_000
G = 256
HID = 64
NEG_SLOPE = 0.2
EPS = 1e-16

NCORES = 8
P = 128            # partitions / edge-tile size / node-block size
RW = 128           # table row width (bf16) -> 256 bytes
C_AS = 64          # a_src column
C_AD = 65          # a_dst column
TCOLS = 66         # meaningful table columns [h | a_src | a_dst]
NQ = 4             # src sub-table quarters
GB = 5             # slots per gather batch
RING = 1024        # SWDGE ring descriptors; the deployed ucode ring is
                   # hard-limited to 1024 (larger rings wedge the device)

F32 = mybir.dt.float32
BF16 = mybir.dt.bfloat16
FP8 = mybir.dt.float8e4
I16 = mybir.dt.int16
U32 = mybir.dt.uint32

BF = ml_dtypes.bfloat16
F8 = ml_dtypes.float8_e4m3


class Prep:
    pass


# ----------------------------------------------------------------------------
# Host-side graph preprocessing
# ----------------------------------------------------------------------------

def _wrap16(flat: np.ndarray) -> np.ndarray:
    """int16 stream -> [128, n/16] wrapped layout (k at [k%16, k//16], x8)."""
    n = flat.shape[0]
    assert n % 16 == 0
    w = flat.reshape(n // 16, 16).T          # [16, n/16]
    return np.tile(w, (8, 1))                # [128, n/16]


def preprocess(edge_index: np.ndarray, edge_attr: np.ndarray, batch: np.ndarray) -> Prep:
    pr = Prep()
    src = edge_index[0].astype(np.int64)
    dst = edge_index[1].astype(np.int64)
    attr = edge_attr[:, 0].astype(np.float32)

    nblk = (N + P - 1) // P
    nblk_pad = ((nblk + NCORES - 1) // NCORES) * NCORES
    n_slots = nblk_pad // NCORES
    n_loc = n_slots * P
    n_tab = NCORES * n_loc
    qrows = n_tab // NQ
    assert qrows <= 32768 and n_tab % NQ == 0

    order = np.argsort(dst, kind="stable")
    dst_s = dst[order]
    src_s = src[order]
    attr_s = attr[order]
    blk_of_edge = dst_s // P
    cnt = np.bincount(blk_of_edge, minlength=nblk_pad)
    seg = np.zeros(nblk_pad + 1, np.int64)
    seg[1:] = np.cumsum(cnt)

    ranked = np.argsort(-cnt, kind="stable")
    block_of = ranked.reshape(n_slots, NCORES)   # [slot, core] -> block

    # node relabel
    new_core = np.full(N, -1, np.int32)
    new_loc = np.full(N, -1, np.int32)
    for s in range(n_slots):
        for c in range(NCORES):
            b = block_of[s, c]
            lo, hi = b * P, min(b * P + P, N)
            if hi <= lo:
                continue
            ids = np.arange(lo, hi)
            new_core[ids] = c
            new_loc[ids] = s * P + (ids - lo)
    assert (new_core >= 0).all()
    new_glob = new_core.astype(np.int64) * n_loc + new_loc.astype(np.int64)

    # per (core, slot, quarter): edge lists (dst-sorted within)
    # and uniform-across-cores tile counts ntq[s][q]
    edges_csq = {}
    cnt_csq = np.zeros((NCORES, n_slots, NQ), np.int64)
    for s in range(n_slots):
        for c in range(NCORES):
            b = block_of[s, c]
            e0, e1 = seg[b], seg[b + 1]
            if e1 <= e0:
                for q in range(NQ):
                    edges_csq[(c, s, q)] = np.empty(0, np.int64)
                continue
            ee = np.arange(e0, e1)
            qq = new_glob[src_s[ee]] // qrows
            for q in range(NQ):
                sel = ee[qq == q]
                edges_csq[(c, s, q)] = sel
                cnt_csq[c, s, q] = sel.shape[0]

    ntq = np.maximum(0, (cnt_csq.max(axis=0) + P - 1) // P)   # [slot, q]
    # ensure every slot has at least one tile
    for s in range(n_slots):
        if ntq[s].sum() == 0:
            ntq[s, 0] = 1
    tb = ntq.sum(axis=1).astype(np.int64)                     # tiles per slot
    tile_off = np.zeros(n_slots + 1, np.int64)
    tile_off[1:] = np.cumsum(tb)
    TT = int(tile_off[-1])

    # per-slot per-tile metadata in slot-tile order (q-major)
    dstl = np.full((NCORES, P, TT), -1.0, np.float32)
    attr_m = np.zeros((NCORES, P, TT), np.float32)
    # int16 index streams
    src16 = np.zeros((NCORES, n_slots, NQ), object)

    for s in range(n_slots):
        for c in range(NCORES):
            jbase = 0
            for q in range(NQ):
                nt = int(ntq[s, q])
                if nt == 0:
                    continue
                ee = edges_csq[(c, s, q)]
                k = ee.shape[0]
                sl = np.zeros(nt * P, np.int16)
                if k:
                    sl[:k] = (new_glob[src_s[ee]] - q * qrows).astype(np.int16)
                    pp = np.arange(k) % P
                    jj = np.arange(k) // P + tile_off[s] + jbase
                    dl = (dst_s[ee] - block_of[s, c] * P).astype(np.float32)
                    dstl[c, pp, jj] = dl
                    attr_m[c, pp, jj] = attr_s[ee]
                src16[c, s, q] = sl
                jbase += nt

    # transposed per-tile dst indicator, fp8: indt[c][n, j*P + p] =
    # (dstl[c, p, j] == n)
    indt = np.zeros((NCORES, P, TT * P), F8)
    for c in range(NCORES):
        d = dstl[c]                       # [P, TT]
        pp, jj = np.nonzero(d >= 0.0)
        nn = d[pp, jj].astype(np.int64)
        blob = np.zeros((P, TT, P), np.uint8)
        blob[nn, jj, pp] = 1
        indt[c] = blob.reshape(P, TT * P).astype(F8)

    # gather-batch index blobs (src only)
    n_gb = (n_slots + GB - 1) // GB
    gb_slots = [list(range(g * GB, min((g + 1) * GB, n_slots))) for g in range(n_gb)]
    sidx_cols = np.zeros((n_gb, NQ), np.int64)   # lengths/16
    for g, sl in enumerate(gb_slots):
        for q in range(NQ):
            sidx_cols[g, q] = sum(int(ntq[s, q]) * P for s in sl) // 16
    sidx_off = np.zeros((n_gb, NQ + 1), np.int64)
    soff = 0
    for g in range(n_gb):
        for q in range(NQ):
            sidx_off[g, q] = soff
            soff += sidx_cols[g, q]
        sidx_off[g, NQ] = soff
    SC = int(soff)

    sidx = np.zeros((NCORES, P, SC), np.int16)
    for c in range(NCORES):
        for g, sl in enumerate(gb_slots):
            for q in range(NQ):
                parts = [src16[c, s, q] for s in sl if int(ntq[s, q])]
                if not parts:
                    continue
                flat = np.concatenate(parts)
                o = int(sidx_off[g, q])
                sidx[c, :, o:o + flat.shape[0] // 16] = _wrap16(flat)

    # graph pooling: per-node graph id and 1/cnt weight, laid out [128, n_slots]
    cnt_g = np.bincount(batch.astype(np.int64), minlength=G).astype(np.float32)
    wg = 1.0 / np.maximum(cnt_g, 1.0)
    g_of = batch.astype(np.int64)
    gid = np.full((NCORES, n_loc), -1.0, np.float32)
    winv = np.zeros((NCORES, n_loc), np.float32)
    ids = np.arange(N)
    gid[new_core[ids], new_loc[ids]] = g_of.astype(np.float32)
    winv[new_core[ids], new_loc[ids]] = wg[g_of]
    gid = gid.reshape(NCORES, n_slots, P).transpose(0, 2, 1)
    winv = winv.reshape(NCORES, n_slots, P).transpose(0, 2, 1)

    pr.n_slots, pr.n_loc, pr.n_tab, pr.TT, pr.qrows = n_slots, n_loc, n_tab, TT, qrows
    pr.ntq, pr.tb, pr.tile_off = ntq, tb, tile_off
    pr.n_gb, pr.gb_slots = n_gb, gb_slots
    pr.sidx_cols, pr.sidx_off = sidx_cols, sidx_off
    pr.SC = SC
    pr.sidx = sidx
    pr.dstl16, pr.attr32 = dstl, attr_m
    pr.indt = indt
    pr.gid, pr.winv = gid, winv
    pr.new_core, pr.new_loc = new_core, new_loc
    return pr


def pack_weights(inputs: dict) -> dict:
    w = {}
    for l in (1, 2, 3):
        W = np.asarray(inputs[f"W{l}"], np.float32)
        a_s = np.asarray(inputs[f"as{l}"], np.float32)
        a_d = np.asarray(inputs[f"ad{l}"], np.float32)
        # x @ wext = [h | h@a_s | h@a_d]
        w[f"wext{l}"] = np.concatenate(
            [W, (W @ a_s)[:, None], (W @ a_d)[:, None]], axis=1)
        w[f"c{l}"] = float(np.asarray(inputs[f"We{l}"], np.float32)[0]
                           @ np.asarray(inputs[f"ae{l}"], np.float32))
        w[f"brep{l}"] = np.tile(np.asarray(inputs[f"b{l}"], np.float32)[None, :], (P, 1))
    w["wlin"] = np.asarray(inputs["Wlin"], np.float32)
    w["blin"] = float(np.asarray(inputs["blin"], np.float32)[0])
    return w


# ----------------------------------------------------------------------------
# Device program
# ----------------------------------------------------------------------------

def build_program(pr: Prep, c_scal):
    n_slots, n_loc, n_tab, TT = pr.n_slots, pr.n_loc, pr.n_tab, pr.TT
    ntq, tb, tile_off = pr.ntq, pr.tb, pr.tile_off

    nc = bacc.Bacc("TRN2", target_bir_lowering=False, debug=False,
                   num_devices=NCORES, dynamic_dma_scratch_size=RING * 16)
    rg = [list(range(NCORES))]

    meta_d = nc.dram_tensor("dstl", [P, TT], F32, kind="ExternalInput")
    attr_d = nc.dram_tensor("attr", [P, TT], F32, kind="ExternalInput")
    sidx_d = nc.dram_tensor("sidx", [P, pr.SC], I16, kind="ExternalInput")
    indt_d = nc.dram_tensor("indt", [P, TT * P], FP8, kind="ExternalInput")
    xT_d = nc.dram_tensor("xT", [HID, n_loc], F32, kind="ExternalInput")
    gid_d = nc.dram_tensor("gid", [P, n_slots], F32, kind="ExternalInput")
    winv_d = nc.dram_tensor("winv", [P, n_slots], F32, kind="ExternalInput")
    wext_d = [nc.dram_tensor(f"wext{l}", [HID, TCOLS], F32, kind="ExternalInput")
              for l in (1, 2, 3)]
    brep_d = [nc.dram_tensor(f"brep{l}", [P, HID], F32, kind="ExternalInput")
              for l in (1, 2, 3)]
    wlin_d = nc.dram_tensor("wlin", [HID, 1], F32, kind="ExternalInput")
    iota_d = nc.dram_tensor("iota", [P, P], BF16, kind="ExternalInput")
    iotg_d = nc.dram_tensor("iotg", [P, G], F32, kind="ExternalInput")
    ident_d = nc.dram_tensor("ident", [P, P], F32, kind="ExternalInput")
    out_d = nc.dram_tensor("out", [P, G // P], F32, kind="ExternalOutput")

    T_full = [nc.dram_tensor(f"T{l}", [n_tab, RW], BF16, kind="Internal",
                             addr_space="Shared") for l in (1, 2, 3)]
    T_sh = [nc.dram_tensor(f"Tsh{l}", [n_loc, RW], BF16, kind="Internal")
            for l in (1, 2, 3)]

    # chunk layout tables (python ints, compile-time)
    chs_src = {}   # (gb, q, s) -> start chunk of that run in hs_src
    ch_src_tot = {}
    for g, sl in enumerate(pr.gb_slots):
        o = 0
        for q in range(NQ):
            for s in sl:
                chs_src[(g, q, s)] = o
                o += int(ntq[s, q])
        ch_src_tot[g] = o
    max_src_ch = max(ch_src_tot.values())
    # indt columns per batch (tiles * P fp8 bytes)
    it_tiles = {g: int(tile_off[sl[-1] + 1] - tile_off[sl[0]])
                for g, sl in enumerate(pr.gb_slots)}
    max_it_tiles = max(it_tiles.values())

    with tile.TileContext(nc) as tc:
        with (
            tc.tile_pool(name="const", bufs=1) as cpool,
            tc.tile_pool(name="sbuf", bufs=4) as spool,
            tc.tile_pool(name="gath", bufs=2) as gpool,
            tc.tile_pool(name="psum", bufs=2, space="PSUM") as ppool,
            tc.tile_pool(name="psumA", bufs=1, space="PSUM") as ppoolA,
            tc.tile_pool(name="psumP", bufs=1, space="PSUM") as ppool1,
            tc.tile_pool(name="psumD", bufs=2, space="PSUM") as ppoolD,
        ):
            iota_sb = cpool.tile([P, P], BF16, tag="iota")
            nc.sync.dma_start(out=iota_sb[:], in_=iota_d[:, :])
            ident_sb = cpool.tile([P, P], F32, tag="ident")
            nc.sync.dma_start(out=ident_sb[:], in_=ident_d[:, :])
            wext_sb, brep_sb = [], []
            for l in range(3):
                t1 = cpool.tile([HID, TCOLS], F32, tag=f"wext{l}", name=f"wext{l}")
                nc.sync.dma_start(out=t1[:], in_=wext_d[l][:, :])
                wext_sb.append(t1)
                t2 = cpool.tile([P, HID], F32, tag=f"brep{l}", name=f"brep{l}")
                nc.sync.dma_start(out=t2[:], in_=brep_d[l][:, :])
                brep_sb.append(t2)
            wlin_sb = cpool.tile([HID, 1], F32, tag="wlin")
            nc.sync.dma_start(out=wlin_sb[:], in_=wlin_d[:, :])
            gid_sb = cpool.tile([P, n_slots], F32, tag="gid")
            nc.sync.dma_start(out=gid_sb[:], in_=gid_d[:, :])
            winv_sb = cpool.tile([P, n_slots], F32, tag="winv")
            nc.sync.dma_start(out=winv_sb[:], in_=winv_d[:, :])
            iotg_sb = cpool.tile([P, G], F32, tag="iotg")
            nc.sync.dma_start(out=iotg_sb[:], in_=iotg_d[:, :])
            ones_sb = cpool.tile([P, 1], BF16, tag="ones")
            nc.vector.memset(ones_sb[:], 1.0)
            # per-layer a_dst columns [128, n_slots] bf16
            adst_sb = [cpool.tile([P, n_slots], BF16, tag=f"adst{l}",
                                  name=f"adst{l}") for l in range(3)]

            # ---- layer-1 table shard (chunked xT loads)
            for g, sl in enumerate(pr.gb_slots):
                s0 = sl[0]
                nsl = len(sl)
                xT_sb = spool.tile([HID, nsl * P], F32, tag="xT",
                                   name=f"xT_{g}")
                nc.sync.dma_start(out=xT_sb[:],
                                  in_=xT_d[:, s0 * P:(s0 + nsl) * P])
                for i, s in enumerate(sl):
                    t_ps = ppoolA.tile([P, TCOLS], F32, tag="tps")
                    nc.tensor.matmul(out=t_ps[:],
                                     lhsT=xT_sb[:, i * P:(i + 1) * P],
                                     rhs=wext_sb[0][:], start=True, stop=True)
                    trow = spool.tile([P, RW], BF16, tag="trow",
                                      name=f"trow0_{s}")
                    nc.scalar.copy(out=trow[:, 0:TCOLS], in_=t_ps[:])
                    nc.vector.memset(trow[:, TCOLS:RW], 0.0)
                    nc.scalar.copy(out=adst_sb[0][:, s:s + 1],
                                   in_=t_ps[:, C_AD:C_AD + 1])
                    nc.sync.dma_start(out=T_sh[0][s * P:(s + 1) * P, :],
                                      in_=trow[:])

            nc.gpsimd.collective_compute(
                "AllGather", mybir.AluOpType.bypass, replica_groups=rg,
                ins=[T_sh[0].ap().opt()], outs=[T_full[0].ap().opt()])

            pool_ps = [ppool1.tile([P, HID], F32, tag=f"pool{h}", name=f"pool{h}")
                       for h in range(G // P)]

            for l in range(3):
                last = l == 2
                for g, sl in enumerate(pr.gb_slots):
                    # ---- src gathers for this batch of slots
                    hs_src = gpool.tile([P, max_src_ch * RW], BF16, tag="hsrc")
                    for q in range(NQ):
                        ncols = int(pr.sidx_cols[g, q])
                        if ncols == 0:
                            continue
                        o = int(pr.sidx_off[g, q])
                        idx_sb = spool.tile([P, ncols], I16, tag="sidx",
                                            name=f"sidx_{l}_{g}_{q}")
                        nc.sync.dma_start(out=idx_sb[:],
                                          in_=sidx_d[:, o:o + ncols])
                        nidx = ncols * 16
                        c0 = chs_src[(g, q, sl[0])]
                        nch = nidx // P
                        # split into <=RING-index pieces
                        maxch = RING // P
                        npieces = (nch + maxch - 1) // maxch
                        for pi in range(npieces):
                            step = (nch + npieces - 1) // npieces
                            ca = pi * step
                            cb = min(nch, (pi + 1) * step)
                            if cb <= ca:
                                continue
                            nc.gpsimd.dma_gather(
                                out_ap=hs_src[:, (c0 + ca) * RW:(c0 + cb) * RW]
                                .rearrange("p (t c) -> p t c", c=RW),
                                in_ap=T_full[l][q * pr.qrows:(q + 1) * pr.qrows, :],
                                idxs_ap=idx_sb[:, ca * 8:cb * 8],
                                num_idxs=(cb - ca) * P,
                                num_idxs_reg=(cb - ca) * P, elem_size=RW)
                    # ---- transposed dst indicators for this batch
                    itt = it_tiles[g]
                    ito = int(tile_off[sl[0]])
                    indt_sb = gpool.tile([P, max_it_tiles * P], FP8, tag="indt")
                    nc.sync.dma_start(
                        out=indt_sb[:, 0:itt * P],
                        in_=indt_d[:, ito * P:(ito + itt) * P])

                    hs3 = hs_src[:].rearrange("p (t c) -> p t c", c=RW)

                    for s in sl:
                        t = int(tb[s])
                        o = int(tile_off[s])
                        dstl_sb = spool.tile([P, t], F32, tag="dstl",
                                             name=f"dstl_{l}_{s}")
                        nc.sync.dma_start(out=dstl_sb[:], in_=meta_d[:, o:o + t])
                        attr_sb = spool.tile([P, t], F32, tag="attrm",
                                             name=f"attr_{l}_{s}")
                        nc.sync.dma_start(out=attr_sb[:], in_=attr_d[:, o:o + t])

                        # a_dst expansion: adst_ps[:, j] = indT_j^T @ adst_col
                        adst_ps = ppoolD.tile([P, t], F32, tag="adstps",
                                              name=f"adstps_{l}_{s}")
                        for j in range(t):
                            col = (o - ito + j) * P
                            # start only once: start_tensor_calc marks the
                            # whole 2KB PSUM zero-region pending-zero
                            nc.tensor.matmul(
                                out=adst_ps[:, j:j + 1],
                                lhsT=indt_sb[:, col:col + P],
                                rhs=adst_sb[l][:, s:s + 1],
                                start=(j == 0), stop=(j == t - 1),
                                skip_group_check=True)

                        # X = a_src[src] + a_dst[dst]
                        X = spool.tile([P, t], F32, tag="xsum", name=f"X_{l}_{s}")
                        jb = 0
                        for q in range(NQ):
                            nt = int(ntq[s, q])
                            if nt == 0:
                                continue
                            cj = chs_src[(g, q, s)]
                            asrc_v = hs3[:, cj:cj + nt, C_AS:C_AS + 1] \
                                .rearrange("p t c -> p (t c)")
                            nc.vector.tensor_tensor(
                                out=X[:, jb:jb + nt], in0=asrc_v,
                                in1=adst_ps[:, jb:jb + nt],
                                op=mybir.AluOpType.add)
                            jb += nt
                        s2 = spool.tile([P, t], F32, tag="s2", name=f"s2_{l}_{s}")
                        nc.vector.scalar_tensor_tensor(
                            out=s2[:], in0=attr_sb[:], scalar=float(c_scal[l]),
                            in1=X[:], op0=mybir.AluOpType.mult,
                            op1=mybir.AluOpType.add)
                        alf = spool.tile([P, t], F32, tag="alf", name=f"alf_{l}_{s}")
                        nc.vector.scalar_tensor_tensor(
                            out=alf[:], in0=s2[:], scalar=NEG_SLOPE,
                            in1=s2[:], op0=mybir.AluOpType.mult,
                            op1=mybir.AluOpType.max)
                        ex = spool.tile([P, t], F32, tag="ex", name=f"ex_{l}_{s}")
                        nc.scalar.activation(out=ex[:], in_=alf[:],
                                             func=mybir.ActivationFunctionType.Exp)

                        # scatter: agg[:, 0:64] = sum ex*h ; agg[:, 64] = sum ex
                        jb = 0
                        agg = ppool.tile([P, C_AS + 1], F32, tag="agg",
                                         name=f"agg_{l}_{s}")
                        first = True
                        for q in range(NQ):
                            nt = int(ntq[s, q])
                            if nt == 0:
                                continue
                            cj = chs_src[(g, q, s)]
                            for k in range(nt):
                                j = jb + k
                                indw = spool.tile([P, P], BF16, tag="indw",
                                                  name=f"iw_{l}_{s}_{j}")
                                nc.vector.tensor_scalar(
                                    out=indw[:], in0=iota_sb[:],
                                    scalar1=dstl_sb[:, j:j + 1],
                                    scalar2=ex[:, j:j + 1],
                                    op0=mybir.AluOpType.is_equal,
                                    op1=mybir.AluOpType.mult)
                                # only the very first matmul starts the bank
                                # (start marks the whole zero-region); the
                                # ones-series accumulates onto pending-zero
                                nc.tensor.matmul(
                                    out=agg[:, 0:C_AS], lhsT=indw[:],
                                    rhs=hs3[:, cj + k, 0:C_AS],
                                    start=first, stop=(j == t - 1),
                                    skip_group_check=True)
                                nc.tensor.matmul(
                                    out=agg[:, C_AS:C_AS + 1], lhsT=indw[:],
                                    rhs=ones_sb[:],
                                    start=False, stop=(j == t - 1),
                                    skip_group_check=True)
                                first = False
                            jb += nt

                        # epilogue
                        dpe = spool.tile([P, 1], F32, tag="dpe", name=f"dpe_{l}_{s}")
                        nc.vector.tensor_scalar_add(
                            out=dpe[:], in0=agg[:, C_AS:C_AS + 1], scalar1=EPS)
                        rcp = spool.tile([P, 1], F32, tag="rcp", name=f"rcp_{l}_{s}")
                        nc.vector.reciprocal(out=rcp[:], in_=dpe[:])
                        x2 = spool.tile([P, HID], F32, tag="x2", name=f"x2_{l}_{s}")
                        nc.scalar.activation(
                            out=x2[:], in_=agg[:, 0:C_AS],
                            func=mybir.ActivationFunctionType.Copy,
                            scale=rcp[:, 0:1])
                        x2b = spool.tile([P, HID], F32, tag="x2b", name=f"x2b_{l}_{s}")
                        nc.vector.tensor_tensor(out=x2b[:], in0=x2[:],
                                                in1=brep_sb[l][:],
                                                op=mybir.AluOpType.add)
                        if not last:
                            x3 = spool.tile([P, HID], F32, tag="x3",
                                            name=f"x3_{l}_{s}")
                            nc.scalar.activation(
                                out=x3[:], in_=x2b[:],
                                func=mybir.ActivationFunctionType.Relu)
                            xt_ps = ppoolA.tile([HID, P], F32, tag="xtps")
                            nc.tensor.transpose(out=xt_ps[:], in_=x3[:],
                                                identity=ident_sb[:])
                            xt_sb = spool.tile([HID, P], F32, tag="xtsb",
                                               name=f"xt_{l}_{s}")
                            nc.scalar.copy(out=xt_sb[:], in_=xt_ps[:])
                            tn_ps = ppoolA.tile([P, TCOLS], F32, tag="tps")
                            nc.tensor.matmul(out=tn_ps[:], lhsT=xt_sb[:],
                                             rhs=wext_sb[l + 1][:],
                                             start=True, stop=True)
                            trow = spool.tile([P, RW], BF16, tag="trow",
                                              name=f"trow_{l}_{s}")
                            nc.scalar.copy(out=trow[:, 0:TCOLS], in_=tn_ps[:])
                            nc.vector.memset(trow[:, TCOLS:RW], 0.0)
                            nc.scalar.copy(out=adst_sb[l + 1][:, s:s + 1],
                                           in_=tn_ps[:, C_AD:C_AD + 1])
                            nc.sync.dma_start(
                                out=T_sh[l + 1][s * P:(s + 1) * P, :],
                                in_=trow[:])
                        else:
                            for h in range(G // P):
                                gih = spool.tile([P, P], F32, tag="gih",
                                                 name=f"gi_{s}_{h}")
                                nc.vector.tensor_scalar(
                                    out=gih[:],
                                    in0=iotg_sb[:, h * P:(h + 1) * P],
                                    scalar1=gid_sb[:, s:s + 1],
                                    scalar2=winv_sb[:, s:s + 1],
                                    op0=mybir.AluOpType.is_equal,
                                    op1=mybir.AluOpType.mult)
                                nc.tensor.matmul(
                                    out=pool_ps[h][:], lhsT=gih[:], rhs=x2b[:],
                                    start=(s == 0), stop=(s == n_slots - 1),
                                    skip_group_check=True)

                if not last:
                    nc.gpsimd.collective_compute(
                        "AllGather", mybir.AluOpType.bypass, replica_groups=rg,
                        ins=[T_sh[l + 1].ap().opt()],
                        outs=[T_full[l + 1].ap().opt()])

            # ---- head
            out_sb = spool.tile([P, G // P], F32, tag="outsb")
            for h in range(G // P):
                pool_sb = spool.tile([P, HID], F32, tag="poolsb",
                                     name=f"poolsb{h}")
                nc.vector.tensor_copy(out=pool_sb[:], in_=pool_ps[h][:])
                pt_ps = ppoolA.tile([HID, P], F32, tag="xtps")
                nc.tensor.transpose(out=pt_ps[:], in_=pool_sb[:],
                                    identity=ident_sb[:])
                pt_sb = spool.tile([HID, P], F32, tag="xtsb", name=f"ptsb{h}")
                nc.scalar.copy(out=pt_sb[:], in_=pt_ps[:])
                o_ps = ppoolA.tile([P, 1], F32, tag="tps", name=f"o_ps{h}")
                nc.tensor.matmul(out=o_ps[:], lhsT=pt_sb[:], rhs=wlin_sb[:],
                                 start=True, stop=True)
                nc.vector.tensor_copy(out=out_sb[:, h:h + 1], in_=o_ps[:])
            nc.sync.dma_start(out=out_d[:, :], in_=out_sb[:])

    nc.compile()
    return nc


# ----------------------------------------------------------------------------
# Entry point
# ----------------------------------------------------------------------------

def make_inmaps(pr: Prep, w: dict, x: np.ndarray):
    iota = np.tile(np.arange(P, dtype=np.float32)[None, :], (P, 1)).astype(BF)
    iotg = np.tile(np.arange(G, dtype=np.float32)[None, :], (P, 1))
    ident = np.eye(P, dtype=np.float32)
    in_maps = []
    for c in range(NCORES):
        xT_c = np.zeros((HID, pr.n_loc), np.float32)
        mask = pr.new_core == c
        xT_c[:, pr.new_loc[mask]] = x[mask].T
        m = {
            "dstl": pr.dstl16[c],
            "attr": pr.attr32[c],
            "sidx": pr.sidx[c],
            "indt": pr.indt[c],
            "xT": xT_c,
            "gid": pr.gid[c],
            "winv": pr.winv[c],
            "wlin": w["wlin"],
            "iota": iota,
            "iotg": iotg,
            "ident": ident,
        }
        for l in (1, 2, 3):
            m[f"wext{l}"] = w[f"wext{l}"]
            m[f"brep{l}"] = w[f"brep{l}"]
        in_maps.append(m)
    return in_maps


def kernel(**inputs) -> np.ndarray:
    inputs = {k: np.asarray(v) for k, v in inputs.items()}
    pr = preprocess(inputs["edge_index"], inputs["edge_attr"], inputs["batch"])
    w = pack_weights(inputs)
    nc = build_program(pr, [w["c1"], w["c2"], w["c3"]])
    in_maps = make_inmaps(pr, w, np.asarray(inputs["x"], np.float32))
    res = bass_utils.run_bass_kernel_spmd(nc, in_maps,
                                          core_ids=list(range(NCORES)))
    out = np.zeros(G, np.float64)
    for c in range(NCORES):
        oc = res.results[c]["out"]
        out += oc.T.reshape(-1).astype(np.float64)
    return (out + w["blin"]).astype(np.float32)
